# revision 36
# baseline (speedup 1.0000x reference)
"""Trainium2 Bass kernel for nn_Attention (B=8, N=1024, C=768, H=12).

Data-parallel over batch: core b handles batch element b.

Math (re-associated to avoid the huge bhqk,bhqd->bkd contraction):
  q = x Wq^T, k = x Wk^T             (per head h: qh, kh  [N, Z])
  S_h = qh kh^T * scale              [N, N]
  E_h = exp(S_h)   (scores are in [-3, 3]; no max-subtraction needed)
  den[qi] = sum_ki E_h[qi, ki]
  ks = kh / den[:, None], qs = qh / den[:, None]
  AT_h = [E_h^T ks ; E_h^T qs]^T     [2Z, N]   (A1T/A2T stacked)
  O    = sum_h [Wq_h ; Wk_h]^T-contracted with AT_h   (outT, [C, N])
  final = O^T Wp^T + bias            (bias added in the PSUM->SBUF copy)

vs the first working version:
  - natural-layout q/k (needed as at-matmul lhsT) comes from PE
    transposes of the qT/kT tiles (12K PE columns) instead of a second
    full projection pass (74K columns).  Head-pair j's natural layout
    is exactly transpose(qT_sb[j]) / transpose(kT_sb[j]).
  - bias is added by the DVE during the final copy (tensor_tensor add
    with a host-replicated [128, C] tile), not by 1-partition matmuls.
  - score matmuls emit head-outer/chunk-inner so consecutive matmuls
    share the stationary operand and exp starts one matmul earlier.
  - LAG=6: at_mm (h,t) emits ~3 t-steps after its exp, shrinking the
    post-pair-5 drain tail and E-tile lifetimes.
  - pair j+1's transposes run at the END of pair j (the S-tile PSUM
    slots are idle across the boundary; mid-pair placement stalls the
    in-order PE queue on the S-slot rotation), and their PSUM->SBUF
    copies run on the Scalar engine, which is idle at the boundary.

Negative results (measured): padding scores to 128-row stationaries
(+9us copy friction for -6us PE), interleaving accumulation groups
across PSUM banks (~2x per-matmul), gpsimd tensor ops (2us each), and
hosting O-combine or transposes as mid-pair extras (S-slot churn).
"""

import sys
from contextlib import ExitStack

import numpy as np

if "/opt/trn_rl_repo" not in sys.path:
    sys.path.insert(0, "/opt/trn_rl_repo")

import ml_dtypes
import concourse.bass as bass
import concourse.mybir as mybir
import concourse.tile as tile
from concourse import bacc, bass_utils
from concourse.bass import ts

B, N, C, H = 8, 1024, 768, 12
Z = C // H          # 64
P = 128
NT = N // P         # 8 qi tiles
CT = C // P         # 6 c tiles
NP = H // 2         # 6 head pairs
SCALE = Z ** -0.5   # 0.125
FP = mybir.dt.float32
BF = mybir.dt.bfloat16
FPR = mybir.dt.float32r

CCH = [(0, 512), (512, 256)]  # C=768 split into matmul free-dim chunks

last_results = None  # set by kernel() for test harness introspection


def _r(ap):
    """bitcast to float32r for full-rate fp32 matmuls (fp32 data only)."""
    if ap.dtype == FP:
        return ap.bitcast(FPR)
    return ap


def emit(ctx: ExitStack, tc: tile.TileContext, io):
    nc = tc.nc
    xT, wqkT, WpT, Wqb, Wkb, bias128, eye, out = io

    stack = []  # (name, free) in creation order; freed strictly LIFO

    def single(shape, dtype, name):
        t, free = tc.tile(shape, dtype, name=name)
        stack.append((name, free))
        return t

    def free_through(name):
        while stack:
            nm, fr = stack.pop()
            fr()
            if nm == name:
                return
        raise KeyError(name)

    # ---------------- PSUM pool: 2 tags x 2 bufs x [128,1024] = 8 banks ----
    psum = ctx.enter_context(tc.tile_pool(name="psum", bufs=2, space="PSUM"))
    _chain = [0]

    def ps_tile(tag=None):
        if tag is None:
            tag = f"ps{_chain[0] & 1}"
            _chain[0] += 1
        return psum.tile([P, N], FP, name=tag, tag=tag)

    # SBUF pools (entered before any single so LIFO holds at ctx exit)
    p_E = ctx.enter_context(tc.tile_pool(name="p_E", bufs=16))
    p_kqs = ctx.enter_context(tc.tile_pool(name="p_kqs", bufs=10))
    p_den = ctx.enter_context(tc.tile_pool(name="p_den", bufs=9))
    p_out = ctx.enter_context(tc.tile_pool(name="p_out", bufs=3))
    p_nat = ctx.enter_context(tc.tile_pool(name="p_nat", bufs=3))

    # ------------- singles, bottom of stack = longest-lived -------------
    W2_all = single([P, H * C], BF, name="W2_all")
    W2_sb = [W2_all[:, ts(h, C)] for h in range(H)]
    AT_sb = [single([P, N], BF, name=f"AT{h}") for h in range(H)]
    bias_sb = single([P, C], FP, name="bias_sb")
    eye_sb = single([P, P], BF, name="eye_sb")
    WpT_all = single([P, CT * C], FPR, name="WpT_all")
    WpT_sb = [WpT_all[:, ts(i, C)] for i in range(CT)]
    # qT/kT tile j: [128, N] rows = c_out 128j..128j+127 (heads 2j, 2j+1)
    qT_sb = [single([P, N], BF, name=f"qT{j}") for j in range(CT)]
    kT_sb = [single([P, N], BF, name=f"kT{j}") for j in range(CT)]
    wqkT_all = single([P, CT * 2 * C], BF, name="wqkT_all")
    wqkT_sb = [wqkT_all[:, ts(i, 2 * C)] for i in range(CT)]
    xT_all = single([P, CT * N], BF, name="xT_all")
    xT_sb = [xT_all[:, ts(i, N)] for i in range(CT)]

    # ---------------- batched input DMAs (phase-A inputs first) ---------
    for k in range(CT):
        nc.sync.dma_start(xT_sb[k][:], xT[ts(k, P), :])
        nc.sync.dma_start(wqkT_sb[k][:], wqkT[ts(k, P), :])
    nc.sync.dma_start(eye_sb[:], eye[:])
    nc.sync.dma_start(W2_all[0:Z, :].rearrange("z (h c) -> z h c", c=C),
                      Wqb.rearrange("(h z) c -> z h c", z=Z))
    nc.sync.dma_start(W2_all[Z:P, :].rearrange("z (h c) -> z h c", c=C),
                      Wkb.rearrange("(h z) c -> z h c", z=Z))
    nc.sync.dma_start(WpT_all[:].rearrange("p (k c) -> p k c", k=CT),
                      WpT.rearrange("(k p) c -> p k c", p=P))
    nc.sync.dma_start(bias_sb[:], bias128[:])

    # ---------------- projection chains ----------------
    # k chains + q-ch0 first: pair j's scores t=0..3 become ready one
    # chain earlier (they read kT fully but only qT cols 0:512)
    QKT_ORDER = [(1, 0), (0, 0), (1, 1), (0, 1)]

    def qkT_mms(j, which, ch, ps, k0, k1, tag=None):
        cols = slice(512 * ch, 512 * ch + 512)
        woff = C * which + 128 * j
        for k in range(k0, k1):
            nc.tensor.matmul(
                ps[:, 0:512],
                lhsT=wqkT_sb[k][:, woff: woff + P],
                rhs=xT_sb[k][:, cols],
                start=(k == 0),
                stop=(k == CT - 1),
            )
        if k1 == CT:
            dst = (qT_sb if which == 0 else kT_sb)[j][:, cols]
            if tag == "pro":  # prologue: ACT is idle, DVE queue is not
                nc.scalar.copy(dst, ps[:, 0:512])
            else:
                nc.vector.tensor_copy(dst, ps[:, 0:512])

    def qkT_chain(j, which, ch, tag=None):
        ps = ps_tile(None if tag == "pro" else tag)
        qkT_mms(j, which, ch, ps, 0, CT, tag)



    # natkq[j]: [128, 2N] bf16.  cols 0:N = k natural, N:2N = q natural;
    # natkq[j][p, g*N + t*128 + cc] = (k,q)[n = t*128 + p, c = j*128 + cc]
    natkq = {}

    def transp_batch(j, which, tag="ps0"):  # which: 0 -> k, 1 -> q
        if j not in natkq:
            natkq[j] = p_nat.tile([P, 2 * N], BF, name="natkq", tag="natkq")
        src = (kT_sb if which == 0 else qT_sb)[j]
        tp = psum.tile([P, N], BF, name="tp", tag=tag)
        # ACT copies (Scalar is idle at pair boundaries, the DVE queue
        # would delay freeing the PSUM slot), split in halves so the slot
        # releases ~0.4us after the last transpose instead of ~0.85us.
        for t in range(NT // 2):
            nc.tensor.transpose(tp[:, ts(t, P)], src[:, ts(t, P)], eye_sb[:])
        nc.scalar.copy(natkq[j][:, which * N: which * N + 512], tp[:, 0:512])
        for t in range(NT // 2, NT):
            nc.tensor.transpose(tp[:, ts(t, P)], src[:, ts(t, P)], eye_sb[:])
        nc.scalar.copy(natkq[j][:, which * N + 512: which * N + N],
                       tp[:, 512:N])

    # qT/kT + natural layout for pair 0 up front
    for which, ch in QKT_ORDER:
        qkT_chain(0, which, ch, "pro")
    transp_batch(0, 0)
    transp_batch(0, 1)

    # ---------------- phase B -------------------------------------------
    at_queue = []
    # Variable at-queue lag: drain ~1 at_mm per t while the chain extras
    # load t=0..3, ~3 per t in the ACT-paced t=4..7 where the PE would
    # otherwise idle ~0.5us/t (at_mms accumulate into ps1, so unlike the
    # chains/transposes they can fill late-t slack without fighting the
    # S tiles for ps0 slots).
    LAGS = [7, 8, 9, 10, 9, 8, 7, 6]

    def drain_at(n):
        while len(at_queue) > n:
            at_queue.pop(0)()

    at_ps = {}

    # extras: one popped per t.  qkT chains for pair j+1 at t=0..3; pair
    # j+1's transposes run at the END of pair j.
    extras = {j: [] for j in range(NP)}
    for j in range(NP - 1):
        for which, ch in QKT_ORDER:
            extras[j].append(
                lambda j=j, which=which, ch=ch: qkT_chain(j + 1, which, ch, "ps0"))

    for j in range(NP):
        heads = (2 * j, 2 * j + 1)
        qt, kt = qT_sb[j], kT_sb[j]
        den_t = {h: p_den.tile([P, NT], FP, name="dent") for h in heads}
        rv_t = {h: p_den.tile([P, NT], FP, name="rvt") for h in heads}
        for h in heads:
            at_ps[h] = ps_tile("ps1")
        ext = extras[j]
        for t in range(NT):
            S = {h: ps_tile("ps0") for h in heads}
            for h in heads:
                base = Z * (h & 1)
                for ch in range(2):
                    cols = slice(512 * ch, 512 * ch + 512)
                    nc.tensor.matmul(
                        S[h][:, cols],
                        lhsT=qt[base:base + Z, ts(t, P)],
                        rhs=kt[base:base + Z, cols],
                        start=True, stop=True,
                    )
            for h in heads:
                E = p_E.tile([P, N], BF, name="Et")
                nc.scalar.activation(
                    E[:], S[h][:], mybir.ActivationFunctionType.Exp,
                    scale=SCALE, accum_out=den_t[h][:, t:t + 1],
                )

                def at_mm(h=h, t=t, j=j, E=E, den_t=den_t, rv_t=rv_t):
                    nc.vector.reciprocal(rv_t[h][:, t:t + 1],
                                         den_t[h][:, t:t + 1])
                    kqs = p_kqs.tile([P, 2 * Z], BF, name="kqst")
                    off = t * P + (h & 1) * Z
                    nc.vector.tensor_scalar_mul(
                        kqs[:].rearrange("p (g z) -> p g z", g=2),
                        natkq[j].rearrange("p (g n) -> p g n", g=2)
                        [:, :, off:off + Z],
                        rv_t[h][:, t:t + 1],
                    )
                    for ch in range(2):
                        cols = slice(512 * ch, 512 * ch + 512)
                        nc.tensor.matmul(
                            at_ps[h][:, cols],
                            lhsT=kqs[:],
                            rhs=E[:, cols],
                            start=(t == 0), stop=(t == NT - 1),
                        )

                at_queue.append(at_mm)
                drain_at(LAGS[t])
            if ext:
                ext.pop(0)()
        if j < NP - 1:
            transp_batch(j + 1, 0)
            transp_batch(j + 1, 1)
        for h in heads:
            def at_copy(h=h):
                nc.vector.tensor_copy(AT_sb[h][:], at_ps.pop(h)[:])
            at_queue.append(at_copy)
    drain_at(0)

    free_through("qT0")  # frees xT, wqkT, kT*, qT*

    # ---------------- phase C: combine over heads, project, bias ------
    O_sb = [single([P, N], FPR, name=f"O{d}") for d in range(CT)]
    for d in range(CT):
        O_ps = ps_tile("ps0")
        for ch in range(2):
            cols = slice(512 * ch, 512 * ch + 512)
            for h in range(H):
                nc.tensor.matmul(
                    O_ps[:, cols],
                    lhsT=W2_sb[h][:, ts(d, P)],
                    rhs=AT_sb[h][:, cols],
                    start=(h == 0), stop=(h == H - 1),
                )
        nc.vector.tensor_copy(O_sb[d][:], O_ps[:])

    for t in range(NT):
        F_ps = ps_tile("ps1")
        for off, w in CCH:
            for k in range(CT):
                nc.tensor.matmul(
                    F_ps[:, off:off + w],
                    lhsT=_r(O_sb[k][:, ts(t, P)]),
                    rhs=_r(WpT_sb[k][:, off:off + w]),
                    start=(k == 0), stop=(k == CT - 1),
                )
        o = p_out.tile([P, C], FP, name="outt")
        nc.vector.tensor_tensor(o[:], F_ps[:, 0:C], bias_sb[:],
                                mybir.AluOpType.add)
        nc.sync.dma_start(out[ts(t, P), :], o[:])

    while stack:
        stack.pop()[1]()


def build():
    nc = bacc.Bacc("TRN2", target_bir_lowering=False, debug=False, num_devices=B)
    xT = nc.dram_tensor("xT", [C, N], BF, kind="ExternalInput").ap()
    wqkT = nc.dram_tensor("wqkT", [C, 2 * C], BF, kind="ExternalInput").ap()
    Wqb = nc.dram_tensor("Wqb", [C, C], BF, kind="ExternalInput").ap()
    Wkb = nc.dram_tensor("Wkb", [C, C], BF, kind="ExternalInput").ap()
    WpT = nc.dram_tensor("WpT", [C, C], FPR, kind="ExternalInput").ap()
    bias128 = nc.dram_tensor("bias128", [P, C], FP, kind="ExternalInput").ap()
    eye = nc.dram_tensor("eye", [P, P], BF, kind="ExternalInput").ap()
    out = nc.dram_tensor("out", [N, C], FP, kind="ExternalOutput").ap()
    with tile.TileContext(nc) as tc, ExitStack() as ctx:
        emit(ctx, tc, (xT, wqkT, WpT, Wqb, Wkb, bias128, eye, out))
    nc.compile()
    return nc


def kernel(x, Wq, Wk, Wp, bp, trace=False, **trace_kwargs):
    global last_results
    x = np.asarray(x, dtype=np.float32)
    Wq = np.asarray(Wq, dtype=np.float32)
    Wk = np.asarray(Wk, dtype=np.float32)
    Wp = np.asarray(Wp, dtype=np.float32)
    bp = np.asarray(bp, dtype=np.float32)

    nc = build()
    bf = ml_dtypes.bfloat16
    wqkTc = np.ascontiguousarray(
        np.concatenate([Wq.T, Wk.T], axis=1)).astype(bf)  # [C, 2C]
    Wqbc = np.ascontiguousarray(Wq).astype(bf)
    Wkbc = np.ascontiguousarray(Wk).astype(bf)
    WpTc = np.ascontiguousarray(Wp.T)                     # [C, C] fp32
    biasc = np.ascontiguousarray(
        np.broadcast_to(bp.reshape(1, C), (P, C)).astype(np.float32))
    eyec = np.eye(P, dtype=bf)
    in_maps = []
    for b in range(B):
        in_maps.append({
            "xT": np.ascontiguousarray(x[b].T).astype(bf),
            "wqkT": wqkTc, "Wqb": Wqbc, "Wkb": Wkbc,
            "WpT": WpTc, "bias128": biasc, "eye": eyec,
        })
    res = bass_utils.run_bass_kernel_spmd(
        nc, in_maps, core_ids=list(range(B)), trace=trace, **trace_kwargs)
    last_results = res
    return np.stack([res.results[b]["out"] for b in range(B)], axis=0)


# revision 37
# speedup vs baseline: 1.0526x; 1.0526x over previous
"""Trainium2 Bass kernel for nn_Attention (B=8, N=1024, C=768, H=12).

Data-parallel over batch: core b handles batch element b.

Math (re-associated to avoid the huge bhqk,bhqd->bkd contraction):
  q = x Wq^T, k = x Wk^T             (per head h: qh, kh  [N, Z])
  S_h = qh kh^T * scale              [N, N]
  E_h = exp(S_h)   (scores are in [-3, 3]; no max-subtraction needed)
  den[qi] = sum_ki E_h[qi, ki]
  ks = kh / den[:, None], qs = qh / den[:, None]
  AT_h = [E_h^T ks ; E_h^T qs]^T     [2Z, N]   (A1T/A2T stacked)
  O    = sum_h [Wq_h ; Wk_h]^T-contracted with AT_h   (outT, [C, N])
  final = O^T Wp^T + bias            (bias added in the PSUM->SBUF copy)

vs the first working version:
  - natural-layout q/k (needed as at-matmul lhsT) comes from PE
    transposes of the qT/kT tiles (12K PE columns) instead of a second
    full projection pass (74K columns).  Head-pair j's natural layout
    is exactly transpose(qT_sb[j]) / transpose(kT_sb[j]).
  - bias is added by the DVE during the final copy (tensor_tensor add
    with a host-replicated [128, C] tile), not by 1-partition matmuls.
  - score matmuls emit head-outer/chunk-inner so consecutive matmuls
    share the stationary operand and exp starts one matmul earlier.
  - LAG=6: at_mm (h,t) emits ~3 t-steps after its exp, shrinking the
    post-pair-5 drain tail and E-tile lifetimes.
  - pair j+1's transposes run at the END of pair j (the S-tile PSUM
    slots are idle across the boundary; mid-pair placement stalls the
    in-order PE queue on the S-slot rotation), and their PSUM->SBUF
    copies run on the Scalar engine, which is idle at the boundary.

Negative results (measured): padding scores to 128-row stationaries
(+9us copy friction for -6us PE), interleaving accumulation groups
across PSUM banks (~2x per-matmul), gpsimd tensor ops (2us each), and
hosting O-combine or transposes as mid-pair extras (S-slot churn).
"""

import sys
from contextlib import ExitStack

import numpy as np

if "/opt/trn_rl_repo" not in sys.path:
    sys.path.insert(0, "/opt/trn_rl_repo")

import ml_dtypes
import concourse.bass as bass
import concourse.mybir as mybir
import concourse.tile as tile
from concourse import bacc, bass_utils
from concourse.bass import ts

B, N, C, H = 8, 1024, 768, 12
Z = C // H          # 64
P = 128
NT = N // P         # 8 qi tiles
CT = C // P         # 6 c tiles
NP = H // 2         # 6 head pairs
SCALE = Z ** -0.5   # 0.125
FP = mybir.dt.float32
BF = mybir.dt.bfloat16
FPR = mybir.dt.float32r

CCH = [(0, 512), (512, 256)]  # C=768 split into matmul free-dim chunks

last_results = None  # set by kernel() for test harness introspection


def _r(ap):
    """bitcast to float32r for full-rate fp32 matmuls (fp32 data only)."""
    if ap.dtype == FP:
        return ap.bitcast(FPR)
    return ap


def emit(ctx: ExitStack, tc: tile.TileContext, io):
    nc = tc.nc
    xT, wqkT, WpT, Wqb, Wkb, bias128, eye, out = io

    stack = []  # (name, free) in creation order; freed strictly LIFO

    def single(shape, dtype, name):
        t, free = tc.tile(shape, dtype, name=name)
        stack.append((name, free))
        return t

    def free_through(name):
        while stack:
            nm, fr = stack.pop()
            fr()
            if nm == name:
                return
        raise KeyError(name)

    # ---------------- PSUM pool: 2 tags x 2 bufs x [128,1024] = 8 banks ----
    psum = ctx.enter_context(tc.tile_pool(name="psum", bufs=2, space="PSUM"))
    _chain = [0]

    def ps_tile(tag=None):
        if tag is None:
            tag = f"ps{_chain[0] & 1}"
            _chain[0] += 1
        return psum.tile([P, N], FP, name=tag, tag=tag)

    # SBUF pools (entered before any single so LIFO holds at ctx exit)
    p_E = ctx.enter_context(tc.tile_pool(name="p_E", bufs=12))
    p_kqs = ctx.enter_context(tc.tile_pool(name="p_kqs", bufs=10))
    p_den = ctx.enter_context(tc.tile_pool(name="p_den", bufs=9))
    p_out = ctx.enter_context(tc.tile_pool(name="p_out", bufs=3))
    p_nat = ctx.enter_context(tc.tile_pool(name="p_nat", bufs=3))

    # ------------- singles, bottom of stack = longest-lived -------------
    W2_all = single([P, H * C], BF, name="W2_all")
    W2_sb = [W2_all[:, ts(h, C)] for h in range(H)]
    AT_sb = [single([P, N], BF, name=f"AT{h}") for h in range(H)]
    bias_sb = single([P, C], FP, name="bias_sb")
    eye_sb = single([P, P], BF, name="eye_sb")
    WpT_all = single([P, CT * C], FPR, name="WpT_all")
    WpT_sb = [WpT_all[:, ts(i, C)] for i in range(CT)]
    # qT/kT tile j: [128, N] rows = c_out 128j..128j+127 (heads 2j, 2j+1)
    qT_sb = [single([P, N], BF, name=f"qT{j}") for j in range(CT)]
    kT_sb = [single([P, N], BF, name=f"kT{j}") for j in range(CT)]
    wqkT_all = single([P, CT * 2 * C], BF, name="wqkT_all")
    wqkT_sb = [wqkT_all[:, ts(i, 2 * C)] for i in range(CT)]
    xT_all = single([P, CT * N], BF, name="xT_all")
    xT_sb = [xT_all[:, ts(i, N)] for i in range(CT)]

    # ---------------- batched input DMAs (phase-A inputs first) ---------
    for k in range(CT):
        nc.sync.dma_start(xT_sb[k][:], xT[ts(k, P), :])
        nc.sync.dma_start(wqkT_sb[k][:], wqkT[ts(k, P), :])
    nc.sync.dma_start(eye_sb[:], eye[:])
    nc.sync.dma_start(W2_all[0:Z, :].rearrange("z (h c) -> z h c", c=C),
                      Wqb.rearrange("(h z) c -> z h c", z=Z))
    nc.sync.dma_start(W2_all[Z:P, :].rearrange("z (h c) -> z h c", c=C),
                      Wkb.rearrange("(h z) c -> z h c", z=Z))
    nc.sync.dma_start(WpT_all[:].rearrange("p (k c) -> p k c", k=CT),
                      WpT.rearrange("(k p) c -> p k c", p=P))
    nc.sync.dma_start(bias_sb[:], bias128[:])

    # ---------------- projection chains ----------------
    def chain(dst_ap, lhsT_of, rhs_of, width, tag=None, act_copy=False):
        """dst_ap = sum_k lhsT_of(k)^T @ rhs_of(k); psum chain + copy."""
        ps = ps_tile(tag)
        for k in range(CT):
            nc.tensor.matmul(
                ps[:, 0:width],
                lhsT=_r(lhsT_of(k)),
                rhs=_r(rhs_of(k)),
                start=(k == 0),
                stop=(k == CT - 1),
            )
        if act_copy:  # prologue: ACT is idle before the first exp
            nc.scalar.copy(dst_ap, ps[:, 0:width])
        else:
            nc.vector.tensor_copy(dst_ap, ps[:, 0:width])

    # k chains + q-ch0 first: pair j's scores t=0..3 become ready one
    # chain earlier (they read kT fully but only qT cols 0:512)
    QKT_ORDER = [(1, 0), (0, 0), (1, 1), (0, 1)]

    def qkT_chain(j, which, ch, tag=None, act_copy=False):
        cols = slice(512 * ch, 512 * ch + 512)
        dst = (qT_sb if which == 0 else kT_sb)[j][:, cols]
        woff = C * which
        chain(dst,
              lambda k: wqkT_sb[k][:, woff + 128 * j: woff + 128 * j + P],
              lambda k: xT_sb[k][:, cols], 512, tag, act_copy)

    # natkq[j]: [128, 2N] bf16.  cols 0:N = k natural, N:2N = q natural;
    # natkq[j][p, g*N + t*128 + cc] = (k,q)[n = t*128 + p, c = j*128 + cc]
    natkq = {}

    def transp_batch(j, which, tag="ps0"):  # which: 0 -> k, 1 -> q
        if j not in natkq:
            natkq[j] = p_nat.tile([P, 2 * N], BF, name="natkq", tag="natkq")
        src = (kT_sb if which == 0 else qT_sb)[j]
        tp = psum.tile([P, N], BF, name="tp", tag=tag)
        # ACT copies (Scalar is idle at pair boundaries, the DVE queue
        # would delay freeing the PSUM slot), split in halves so the slot
        # releases ~0.4us after the last transpose instead of ~0.85us.
        for t in range(NT // 2):
            nc.tensor.transpose(tp[:, ts(t, P)], src[:, ts(t, P)], eye_sb[:])
        nc.scalar.copy(natkq[j][:, which * N: which * N + 512], tp[:, 0:512])
        for t in range(NT // 2, NT):
            nc.tensor.transpose(tp[:, ts(t, P)], src[:, ts(t, P)], eye_sb[:])
        nc.scalar.copy(natkq[j][:, which * N + 512: which * N + N],
                       tp[:, 512:N])

    # qT/kT + natural layout for pair 0 up front
    for which, ch in QKT_ORDER:
        qkT_chain(0, which, ch, act_copy=True)
    transp_batch(0, 0)
    transp_batch(0, 1)

    # ---------------- phase B -------------------------------------------
    at_queue = []
    LAG = 6  # at_mm (h,t) emits ~3 t-steps after its exp

    def drain_at(n):
        while len(at_queue) > n:
            at_queue.pop(0)()

    at_ps = {}

    # extras: one popped per t.  qkT chains for pair j+1 at t=0..3; pair
    # j+1's transposes run at the END of pair j.
    extras = {j: [] for j in range(NP)}
    for j in range(NP - 1):
        for which, ch in QKT_ORDER:
            extras[j].append(
                lambda j=j, which=which, ch=ch: qkT_chain(j + 1, which, ch, "ps0"))

    for j in range(NP):
        heads = (2 * j, 2 * j + 1)
        qt, kt = qT_sb[j], kT_sb[j]
        den_t = {h: p_den.tile([P, NT], FP, name="dent") for h in heads}
        rv_t = {h: p_den.tile([P, NT], FP, name="rvt") for h in heads}
        for h in heads:
            at_ps[h] = ps_tile("ps1")
        ext = extras[j]
        for t in range(NT):
            S = {h: ps_tile("ps0") for h in heads}
            for h in heads:
                base = Z * (h & 1)
                for ch in range(2):
                    cols = slice(512 * ch, 512 * ch + 512)
                    nc.tensor.matmul(
                        S[h][:, cols],
                        lhsT=qt[base:base + Z, ts(t, P)],
                        rhs=kt[base:base + Z, cols],
                        start=True, stop=True,
                    )
            for h in heads:
                E = p_E.tile([P, N], BF, name="Et")
                nc.scalar.activation(
                    E[:], S[h][:], mybir.ActivationFunctionType.Exp,
                    scale=SCALE, accum_out=den_t[h][:, t:t + 1],
                )

                def at_mm(h=h, t=t, j=j, E=E, den_t=den_t, rv_t=rv_t):
                    nc.vector.reciprocal(rv_t[h][:, t:t + 1],
                                         den_t[h][:, t:t + 1])
                    kqs = p_kqs.tile([P, 2 * Z], BF, name="kqst")
                    off = t * P + (h & 1) * Z
                    nc.vector.tensor_scalar_mul(
                        kqs[:].rearrange("p (g z) -> p g z", g=2),
                        natkq[j].rearrange("p (g n) -> p g n", g=2)
                        [:, :, off:off + Z],
                        rv_t[h][:, t:t + 1],
                    )
                    for ch in range(2):
                        cols = slice(512 * ch, 512 * ch + 512)
                        nc.tensor.matmul(
                            at_ps[h][:, cols],
                            lhsT=kqs[:],
                            rhs=E[:, cols],
                            start=(t == 0), stop=(t == NT - 1),
                        )

                at_queue.append(at_mm)
                drain_at(LAG)
            if ext:
                ext.pop(0)()
        if j < NP - 1:
            transp_batch(j + 1, 0)
            transp_batch(j + 1, 1)
        for h in heads:
            def at_copy(h=h):
                nc.vector.tensor_copy(AT_sb[h][:], at_ps.pop(h)[:])
            at_queue.append(at_copy)
    drain_at(0)

    free_through("qT0")  # frees xT, wqkT, kT*, qT*

    # ---------------- phase C: combine over heads, project, bias ------
    O_sb = [single([P, N], FPR, name=f"O{d}") for d in range(CT)]
    for d in range(CT):
        O_ps = ps_tile("ps0")
        for ch in range(2):
            cols = slice(512 * ch, 512 * ch + 512)
            for h in range(H):
                nc.tensor.matmul(
                    O_ps[:, cols],
                    lhsT=W2_sb[h][:, ts(d, P)],
                    rhs=AT_sb[h][:, cols],
                    start=(h == 0), stop=(h == H - 1),
                )
        nc.vector.tensor_copy(O_sb[d][:], O_ps[:])

    for t in range(NT):
        F_ps = ps_tile("ps1")
        for off, w in CCH:
            for k in range(CT):
                nc.tensor.matmul(
                    F_ps[:, off:off + w],
                    lhsT=_r(O_sb[k][:, ts(t, P)]),
                    rhs=_r(WpT_sb[k][:, off:off + w]),
                    start=(k == 0), stop=(k == CT - 1),
                )
        o = p_out.tile([P, C], FP, name="outt")
        nc.vector.tensor_tensor(o[:], F_ps[:, 0:C], bias_sb[:],
                                mybir.AluOpType.add)
        nc.sync.dma_start(out[ts(t, P), :], o[:])

    while stack:
        stack.pop()[1]()


def build():
    nc = bacc.Bacc("TRN2", target_bir_lowering=False, debug=False, num_devices=B)
    xT = nc.dram_tensor("xT", [C, N], BF, kind="ExternalInput").ap()
    wqkT = nc.dram_tensor("wqkT", [C, 2 * C], BF, kind="ExternalInput").ap()
    Wqb = nc.dram_tensor("Wqb", [C, C], BF, kind="ExternalInput").ap()
    Wkb = nc.dram_tensor("Wkb", [C, C], BF, kind="ExternalInput").ap()
    WpT = nc.dram_tensor("WpT", [C, C], FPR, kind="ExternalInput").ap()
    bias128 = nc.dram_tensor("bias128", [P, C], FP, kind="ExternalInput").ap()
    eye = nc.dram_tensor("eye", [P, P], BF, kind="ExternalInput").ap()
    out = nc.dram_tensor("out", [N, C], FP, kind="ExternalOutput").ap()
    with tile.TileContext(nc) as tc, ExitStack() as ctx:
        emit(ctx, tc, (xT, wqkT, WpT, Wqb, Wkb, bias128, eye, out))
    nc.compile()
    return nc


def kernel(x, Wq, Wk, Wp, bp, trace=False, **trace_kwargs):
    global last_results
    x = np.asarray(x, dtype=np.float32)
    Wq = np.asarray(Wq, dtype=np.float32)
    Wk = np.asarray(Wk, dtype=np.float32)
    Wp = np.asarray(Wp, dtype=np.float32)
    bp = np.asarray(bp, dtype=np.float32)

    nc = build()
    bf = ml_dtypes.bfloat16
    wqkTc = np.ascontiguousarray(
        np.concatenate([Wq.T, Wk.T], axis=1)).astype(bf)  # [C, 2C]
    Wqbc = np.ascontiguousarray(Wq).astype(bf)
    Wkbc = np.ascontiguousarray(Wk).astype(bf)
    WpTc = np.ascontiguousarray(Wp.T)                     # [C, C] fp32
    biasc = np.ascontiguousarray(
        np.broadcast_to(bp.reshape(1, C), (P, C)).astype(np.float32))
    eyec = np.eye(P, dtype=bf)
    in_maps = []
    for b in range(B):
        in_maps.append({
            "xT": np.ascontiguousarray(x[b].T).astype(bf),
            "wqkT": wqkTc, "Wqb": Wqbc, "Wkb": Wkbc,
            "WpT": WpTc, "bias128": biasc, "eye": eyec,
        })
    res = bass_utils.run_bass_kernel_spmd(
        nc, in_maps, core_ids=list(range(B)), trace=trace, **trace_kwargs)
    last_results = res
    return np.stack([res.results[b]["out"] for b in range(B)], axis=0)


# revision 38
# speedup vs baseline: 1.0988x; 1.0439x over previous
"""Trainium2 Bass kernel for nn_Attention (B=8, N=1024, C=768, H=12).

Data-parallel over batch: core b handles batch element b.

Math (re-associated to avoid the huge bhqk,bhqd->bkd contraction):
  q = x Wq^T, k = x Wk^T             (per head h: qh, kh  [N, Z])
  S_h = qh kh^T * scale              [N, N]
  E_h = exp(S_h)   (scores are in [-3, 3]; no max-subtraction needed)
  den[qi] = sum_ki E_h[qi, ki]
  ks = kh / den[:, None], qs = qh / den[:, None]
  AT_h = [E_h^T ks ; E_h^T qs]^T     [2Z, N]   (A1T/A2T stacked)
  O    = sum_h [Wq_h ; Wk_h]^T-contracted with AT_h   (outT, [C, N])
  final = O^T Wp^T + bias            (bias added in the PSUM->SBUF copy)

vs the first working version:
  - natural-layout q/k (needed as at-matmul lhsT) comes from PE
    transposes of the qT/kT tiles (12K PE columns) instead of a second
    full projection pass (74K columns).  Head-pair j's natural layout
    is exactly transpose(qT_sb[j]) / transpose(kT_sb[j]).
  - bias is added by the DVE during the final copy (tensor_tensor add
    with a host-replicated [128, C] tile), not by 1-partition matmuls.
  - score matmuls emit head-outer/chunk-inner so consecutive matmuls
    share the stationary operand and exp starts one matmul earlier.
  - LAG=6: at_mm (h,t) emits ~3 t-steps after its exp, shrinking the
    post-pair-5 drain tail and E-tile lifetimes.
  - pair j+1's transposes run at the END of pair j (the S-tile PSUM
    slots are idle across the boundary; mid-pair placement stalls the
    in-order PE queue on the S-slot rotation), and their PSUM->SBUF
    copies run on the Scalar engine, which is idle at the boundary.

Negative results (measured): padding scores to 128-row stationaries
(+9us copy friction for -6us PE), interleaving accumulation groups
across PSUM banks (~2x per-matmul), gpsimd tensor ops (2us each), and
hosting O-combine or transposes as mid-pair extras (S-slot churn).
"""

import sys
from contextlib import ExitStack

import numpy as np

if "/opt/trn_rl_repo" not in sys.path:
    sys.path.insert(0, "/opt/trn_rl_repo")

import ml_dtypes
import concourse.bass as bass
import concourse.mybir as mybir
import concourse.tile as tile
from concourse import bacc, bass_utils
from concourse.bass import ts

B, N, C, H = 8, 1024, 768, 12
Z = C // H          # 64
P = 128
NT = N // P         # 8 qi tiles
CT = C // P         # 6 c tiles
NP = H // 2         # 6 head pairs
SCALE = Z ** -0.5   # 0.125
FP = mybir.dt.float32
BF = mybir.dt.bfloat16
FPR = mybir.dt.float32r

CCH = [(0, 512), (512, 256)]  # C=768 split into matmul free-dim chunks

last_results = None  # set by kernel() for test harness introspection


def _r(ap):
    """bitcast to float32r for full-rate fp32 matmuls (fp32 data only)."""
    if ap.dtype == FP:
        return ap.bitcast(FPR)
    return ap


def emit(ctx: ExitStack, tc: tile.TileContext, io):
    nc = tc.nc
    xT, wqkT, WpT, Wqb, Wkb, bias128, eye, out = io

    stack = []  # (name, free) in creation order; freed strictly LIFO

    def single(shape, dtype, name):
        t, free = tc.tile(shape, dtype, name=name)
        stack.append((name, free))
        return t

    def free_through(name):
        while stack:
            nm, fr = stack.pop()
            fr()
            if nm == name:
                return
        raise KeyError(name)

    # ---------------- PSUM pool: 2 tags x 2 bufs x [128,1024] = 8 banks ----
    psum = ctx.enter_context(tc.tile_pool(name="psum", bufs=2, space="PSUM"))
    _chain = [0]

    def ps_tile(tag=None):
        if tag is None:
            tag = f"ps{_chain[0] & 1}"
            _chain[0] += 1
        return psum.tile([P, N], FP, name=tag, tag=tag)

    # SBUF pools (entered before any single so LIFO holds at ctx exit)
    p_E = ctx.enter_context(tc.tile_pool(name="p_E", bufs=16))
    p_kqs = ctx.enter_context(tc.tile_pool(name="p_kqs", bufs=10))
    p_den = ctx.enter_context(tc.tile_pool(name="p_den", bufs=9))
    p_out = ctx.enter_context(tc.tile_pool(name="p_out", bufs=3))
    p_nat = ctx.enter_context(tc.tile_pool(name="p_nat", bufs=3))

    # ------------- singles, bottom of stack = longest-lived -------------
    W2_all = single([P, H * C], BF, name="W2_all")
    W2_sb = [W2_all[:, ts(h, C)] for h in range(H)]
    AT_sb = [single([P, N], BF, name=f"AT{h}") for h in range(H)]
    bias_sb = single([P, C], FP, name="bias_sb")
    eye_sb = single([P, P], BF, name="eye_sb")
    WpT_all = single([P, CT * C], FPR, name="WpT_all")
    WpT_sb = [WpT_all[:, ts(i, C)] for i in range(CT)]
    # qT/kT tile j: [128, N] rows = c_out 128j..128j+127 (heads 2j, 2j+1)
    qT_sb = [single([P, N], BF, name=f"qT{j}") for j in range(CT)]
    kT_sb = [single([P, N], BF, name=f"kT{j}") for j in range(CT)]
    wqkT_all = single([P, CT * 2 * C], BF, name="wqkT_all")
    wqkT_sb = [wqkT_all[:, ts(i, 2 * C)] for i in range(CT)]
    xT_all = single([P, CT * N], BF, name="xT_all")
    xT_sb = [xT_all[:, ts(i, N)] for i in range(CT)]

    # ---------------- batched input DMAs (phase-A inputs first) ---------
    for k in range(CT):
        nc.sync.dma_start(xT_sb[k][:], xT[ts(k, P), :])
        nc.sync.dma_start(wqkT_sb[k][:], wqkT[ts(k, P), :])
    nc.sync.dma_start(eye_sb[:], eye[:])
    nc.sync.dma_start(W2_all[0:Z, :].rearrange("z (h c) -> z h c", c=C),
                      Wqb.rearrange("(h z) c -> z h c", z=Z))
    nc.sync.dma_start(W2_all[Z:P, :].rearrange("z (h c) -> z h c", c=C),
                      Wkb.rearrange("(h z) c -> z h c", z=Z))
    nc.sync.dma_start(WpT_all[:].rearrange("p (k c) -> p k c", k=CT),
                      WpT.rearrange("(k p) c -> p k c", p=P))
    nc.sync.dma_start(bias_sb[:], bias128[:])

    # ---------------- projection chains ----------------
    def chain(dst_ap, lhsT_of, rhs_of, width, tag=None):
        """dst_ap = sum_k lhsT_of(k)^T @ rhs_of(k); psum chain + DVE copy."""
        ps = ps_tile(tag)
        for k in range(CT):
            nc.tensor.matmul(
                ps[:, 0:width],
                lhsT=_r(lhsT_of(k)),
                rhs=_r(rhs_of(k)),
                start=(k == 0),
                stop=(k == CT - 1),
            )
        nc.vector.tensor_copy(dst_ap, ps[:, 0:width])

    # k chains + q-ch0 first: pair j's scores t=0..3 become ready one
    # chain earlier (they read kT fully but only qT cols 0:512)
    QKT_ORDER = [(1, 0), (0, 0), (1, 1), (0, 1)]

    def qkT_chain(j, which, ch, tag=None):
        cols = slice(512 * ch, 512 * ch + 512)
        dst = (qT_sb if which == 0 else kT_sb)[j][:, cols]
        woff = C * which
        chain(dst,
              lambda k: wqkT_sb[k][:, woff + 128 * j: woff + 128 * j + P],
              lambda k: xT_sb[k][:, cols], 512, tag)

    # natkq[j]: [128, 2N] bf16.  cols 0:N = k natural, N:2N = q natural;
    # natkq[j][p, g*N + t*128 + cc] = (k,q)[n = t*128 + p, c = j*128 + cc]
    natkq = {}

    def transp_batch(j, which, tag="ps0"):  # which: 0 -> k, 1 -> q
        if j not in natkq:
            natkq[j] = p_nat.tile([P, 2 * N], BF, name="natkq", tag="natkq")
        src = (kT_sb if which == 0 else qT_sb)[j]
        tp = psum.tile([P, N], BF, name="tp", tag=tag)
        for t in range(NT):
            nc.tensor.transpose(tp[:, ts(t, P)], src[:, ts(t, P)], eye_sb[:])
        # ACT copy: the Scalar engine is idle at pair boundaries, while the
        # DVE queue would delay freeing the PSUM slot for the next S tiles.
        nc.scalar.copy(natkq[j][:, which * N: which * N + N], tp[:])

    # qT/kT + natural layout for pair 0 up front
    for which, ch in QKT_ORDER:
        qkT_chain(0, which, ch)
    transp_batch(0, 0)
    transp_batch(0, 1)

    # ---------------- phase B -------------------------------------------
    at_queue = []
    LAG = 6  # at_mm (h,t) emits ~3 t-steps after its exp

    def drain_at(n):
        while len(at_queue) > n:
            at_queue.pop(0)()

    at_ps = {}

    # extras: one popped per t.  qkT chains for pair j+1 at t=0..3; pair
    # j+1's transposes run at the END of pair j.
    extras = {j: [] for j in range(NP)}
    for j in range(NP - 1):
        for which, ch in QKT_ORDER:
            extras[j].append(
                lambda j=j, which=which, ch=ch: qkT_chain(j + 1, which, ch, "ps0"))

    for j in range(NP):
        heads = (2 * j, 2 * j + 1)
        qt, kt = qT_sb[j], kT_sb[j]
        den_t = {h: p_den.tile([P, NT], FP, name="dent") for h in heads}
        rv_t = {h: p_den.tile([P, NT], FP, name="rvt") for h in heads}
        for h in heads:
            at_ps[h] = ps_tile("ps1")
        ext = extras[j]
        for t in range(NT):
            S = {h: ps_tile("ps0") for h in heads}
            for h in heads:
                base = Z * (h & 1)
                for ch in range(2):
                    cols = slice(512 * ch, 512 * ch + 512)
                    nc.tensor.matmul(
                        S[h][:, cols],
                        lhsT=qt[base:base + Z, ts(t, P)],
                        rhs=kt[base:base + Z, cols],
                        start=True, stop=True,
                    )
            for h in heads:
                E = p_E.tile([P, N], BF, name="Et")
                nc.scalar.activation(
                    E[:], S[h][:], mybir.ActivationFunctionType.Exp,
                    scale=SCALE, accum_out=den_t[h][:, t:t + 1],
                )

                def at_mm(h=h, t=t, j=j, E=E, den_t=den_t, rv_t=rv_t):
                    nc.vector.reciprocal(rv_t[h][:, t:t + 1],
                                         den_t[h][:, t:t + 1])
                    kqs = p_kqs.tile([P, 2 * Z], BF, name="kqst")
                    off = t * P + (h & 1) * Z
                    nc.vector.tensor_scalar_mul(
                        kqs[:].rearrange("p (g z) -> p g z", g=2),
                        natkq[j].rearrange("p (g n) -> p g n", g=2)
                        [:, :, off:off + Z],
                        rv_t[h][:, t:t + 1],
                    )
                    for ch in range(2):
                        cols = slice(512 * ch, 512 * ch + 512)
                        nc.tensor.matmul(
                            at_ps[h][:, cols],
                            lhsT=kqs[:],
                            rhs=E[:, cols],
                            start=(t == 0), stop=(t == NT - 1),
                        )

                at_queue.append(at_mm)
                drain_at(LAG)
            if ext:
                ext.pop(0)()
        if j < NP - 1:
            transp_batch(j + 1, 0)
            transp_batch(j + 1, 1)
        for h in heads:
            def at_copy(h=h):
                nc.vector.tensor_copy(AT_sb[h][:], at_ps.pop(h)[:])
            at_queue.append(at_copy)
    drain_at(0)

    free_through("qT0")  # frees xT, wqkT, kT*, qT*

    # ---------------- phase C: combine over heads, project, bias ------
    O_sb = [single([P, N], FPR, name=f"O{d}") for d in range(CT)]
    for d in range(CT):
        O_ps = ps_tile("ps0")
        for ch in range(2):
            cols = slice(512 * ch, 512 * ch + 512)
            for h in range(H):
                nc.tensor.matmul(
                    O_ps[:, cols],
                    lhsT=W2_sb[h][:, ts(d, P)],
                    rhs=AT_sb[h][:, cols],
                    start=(h == 0), stop=(h == H - 1),
                )
        nc.vector.tensor_copy(O_sb[d][:], O_ps[:])

    for t in range(NT):
        F_ps = ps_tile("ps1")
        for off, w in CCH:
            for k in range(CT):
                nc.tensor.matmul(
                    F_ps[:, off:off + w],
                    lhsT=_r(O_sb[k][:, ts(t, P)]),
                    rhs=_r(WpT_sb[k][:, off:off + w]),
                    start=(k == 0), stop=(k == CT - 1),
                )
        o = p_out.tile([P, C], FP, name="outt")
        if t == NT - 1:
            # last tile: halve the add+DMA so the epilogue overlaps
            nc.vector.tensor_tensor(o[:, 0:384], F_ps[:, 0:384],
                                    bias_sb[:, 0:384], mybir.AluOpType.add)
            nc.sync.dma_start(out[ts(t, P), 0:384], o[:, 0:384])
            nc.vector.tensor_tensor(o[:, 384:C], F_ps[:, 384:C],
                                    bias_sb[:, 384:C], mybir.AluOpType.add)
            nc.sync.dma_start(out[ts(t, P), 384:C], o[:, 384:C])
        else:
            nc.vector.tensor_tensor(o[:], F_ps[:, 0:C], bias_sb[:],
                                    mybir.AluOpType.add)
            nc.sync.dma_start(out[ts(t, P), :], o[:])

    while stack:
        stack.pop()[1]()


def build():
    nc = bacc.Bacc("TRN2", target_bir_lowering=False, debug=False, num_devices=B)
    xT = nc.dram_tensor("xT", [C, N], BF, kind="ExternalInput").ap()
    wqkT = nc.dram_tensor("wqkT", [C, 2 * C], BF, kind="ExternalInput").ap()
    Wqb = nc.dram_tensor("Wqb", [C, C], BF, kind="ExternalInput").ap()
    Wkb = nc.dram_tensor("Wkb", [C, C], BF, kind="ExternalInput").ap()
    WpT = nc.dram_tensor("WpT", [C, C], FPR, kind="ExternalInput").ap()
    bias128 = nc.dram_tensor("bias128", [P, C], FP, kind="ExternalInput").ap()
    eye = nc.dram_tensor("eye", [P, P], BF, kind="ExternalInput").ap()
    out = nc.dram_tensor("out", [N, C], FP, kind="ExternalOutput").ap()
    with tile.TileContext(nc) as tc, ExitStack() as ctx:
        emit(ctx, tc, (xT, wqkT, WpT, Wqb, Wkb, bias128, eye, out))
    nc.compile()
    return nc


def kernel(x, Wq, Wk, Wp, bp, trace=False, **trace_kwargs):
    global last_results
    x = np.asarray(x, dtype=np.float32)
    Wq = np.asarray(Wq, dtype=np.float32)
    Wk = np.asarray(Wk, dtype=np.float32)
    Wp = np.asarray(Wp, dtype=np.float32)
    bp = np.asarray(bp, dtype=np.float32)

    nc = build()
    bf = ml_dtypes.bfloat16
    wqkTc = np.ascontiguousarray(
        np.concatenate([Wq.T, Wk.T], axis=1)).astype(bf)  # [C, 2C]
    Wqbc = np.ascontiguousarray(Wq).astype(bf)
    Wkbc = np.ascontiguousarray(Wk).astype(bf)
    WpTc = np.ascontiguousarray(Wp.T)                     # [C, C] fp32
    biasc = np.ascontiguousarray(
        np.broadcast_to(bp.reshape(1, C), (P, C)).astype(np.float32))
    eyec = np.eye(P, dtype=bf)
    in_maps = []
    for b in range(B):
        in_maps.append({
            "xT": np.ascontiguousarray(x[b].T).astype(bf),
            "wqkT": wqkTc, "Wqb": Wqbc, "Wkb": Wkbc,
            "WpT": WpTc, "bias128": biasc, "eye": eyec,
        })
    res = bass_utils.run_bass_kernel_spmd(
        nc, in_maps, core_ids=list(range(B)), trace=trace, **trace_kwargs)
    last_results = res
    return np.stack([res.results[b]["out"] for b in range(B)], axis=0)
